# revision 3
# baseline (speedup 1.0000x reference)
"""Trainium2 Bass kernel for nn_Connector_77738908057780 (dense_mlp).

Computation (see reference):
  x   = image_features                      [B, N, H]    bf16
  f1  = mean(hidden[0:13],  axis=0)         [B, N, H]
  f2  = mean(hidden[13:26], axis=0)         [B, N, H]
  cat = concat([x, f1, f2], -1)             [B, N, 3H]
  h   = gelu(cat @ W1.T + b1)               W1 = nf4_dequant(codes1, scales1) [H, 3H]
  fg  = h @ W2.T + b2                       W2 = nf4_dequant(codes2, scales2) [H, H]
  out = w * LN(fg) + (1-w) * LN(x),         w = sigmoid(alpha)

Sharding: data-parallel over batch B=8 -> one batch element per NeuronCore.

Per-core plan (v2 -- chunked pipeline):
  - 6 token chunks (5x128 + 1x89).  The 26-layer `hidden` stream dominates
    HBM traffic (43.7 MB/core); it is issued as 12 large 3.8 MB DMAs on the
    sync HWDGE queue in chunk order so DMA stays saturated end-to-end.
    Weights stream on the scalar queue behind chunk 0's loads.
  - layer sums: DVE tensor_add chains (bf16 2x mode), 4 adds offloaded to
    GPSIMD to keep DVE under the per-chunk DMA budget.
  - cat^T is never materialized: GEMM1's k-loop reads x^T (host-transposed
    input), s1^T and s2^T (TensorE identity-transpose -> PSUM -> ACT copy)
    as three separate SBUF tiles.  No SBUF->SBUF xbar DMA at all.
  - GEMM1 weights-stationary -> h^T in PSUM; GELU(+b1 per-partition bias) on
    ACT -> g^T feeds GEMM2 as stationary; b2 is added by a rank-1 matmul
    (ones-row x b2-row) inside the accumulation group; ACT drains PSUM->fg
    while computing sum(fg) via accum_out.
  - LN stats: ACT accum_out gives S(v), S(v^2); DVE combines to mean/var,
    reciprocal+sqrt for rsqrt; 4 fused scalar_tensor_tensor ops for the
    normalize + sigmoid-gate combine.

NF4 dequant of the (small, replicated) weights is host-side weight prep; the
bf16 weights are less DMA traffic than the int32 codes.
"""

import os
import sys

import numpy as np
import ml_dtypes

for _p in ("/opt/trn_rl_repo", "/root/.axon_site/_ro/trn_rl_repo"):
    if os.path.isdir(_p) and _p not in sys.path:
        sys.path.insert(0, _p)

import concourse.bass as bass
import concourse.mybir as mybir
import concourse.tile as tile
from concourse import bacc
from concourse import bass_utils

BF16 = mybir.dt.bfloat16
F32 = mybir.dt.float32
AF = mybir.ActivationFunctionType
ALU = mybir.AluOpType

NP_BF16 = ml_dtypes.bfloat16

P = 128
H = 1152
H3 = 3456
NT = 729          # tokens per core (N); B=8 cores
L = 26
KO1 = H3 // P     # 27 k-tiles for GEMM1
KO2 = H // P      # 9 k-tiles for GEMM2
MO = H // P       # 9 output-feature tiles
EPS = 1e-5
NCHUNK = 3        # fg free-dim chunks of 384
CH = H // NCHUNK  # 384

# Token chunks: 5 full 128-token chunks + one 89-token tail chunk.
CHUNKS = [(0, 128), (128, 128), (256, 128), (384, 128), (512, 128), (640, 89)]
NCH = len(CHUNKS)
NTP = NCH * P     # 768, token count padded for the x^T host layout

NF4_CODEBOOK = np.array([
    -1.0, -0.6961928009986877, -0.5250730514526367, -0.39491748809814453,
    -0.28444138169288635, -0.18477343022823334, -0.09105003625154495, 0.0,
    0.07958029955625534, 0.16093020141124725, 0.24611230194568634,
    0.33791524171829224, 0.4407098591327667, 0.5626170039176941,
    0.7229568362236023, 1.0], dtype=np.float32)

BLOCK = 64


def _dequant_nf4(codes, scales):
    """Match reference: codebook lookup * per-64-block absmax, cast bf16."""
    out_f, in_f = codes.shape
    w = NF4_CODEBOOK[codes].reshape(out_f, in_f // BLOCK, BLOCK)
    w = w * scales[:, :, None].astype(np.float32)
    return w.reshape(out_f, in_f)  # float32 (caller casts)


def _build_program(act=AF.Gelu):
    nc = bacc.Bacc(
        "TRN2",
        target_bir_lowering=False,
        debug=False,
        num_devices=1,
    )
    x_d = nc.dram_tensor("x", (NT, H), BF16, kind="ExternalInput").ap()
    xtc_d = nc.dram_tensor("xtc", (NCH, P, MO, P), BF16, kind="ExternalInput").ap()
    hid_d = nc.dram_tensor("hid", (L, NT, H), BF16, kind="ExternalInput").ap()
    w1t_d = nc.dram_tensor("w1t", (H3, H), BF16, kind="ExternalInput").ap()
    w2t_d = nc.dram_tensor("w2t", (H, H), BF16, kind="ExternalInput").ap()
    b1s_d = nc.dram_tensor("b1s", (P, MO), F32, kind="ExternalInput").ap()
    b2s_d = nc.dram_tensor("b2s", (1, H), BF16, kind="ExternalInput").ap()
    ident_d = nc.dram_tensor("ident", (P, P), BF16, kind="ExternalInput").ap()
    g1b_d = nc.dram_tensor("g1b", (P, H), BF16, kind="ExternalInput").ap()
    g2b_d = nc.dram_tensor("g2b", (P, H), BF16, kind="ExternalInput").ap()
    bcb_d = nc.dram_tensor("bcb", (P, H), BF16, kind="ExternalInput").ap()
    out_d = nc.dram_tensor("out", (NT, H), BF16, kind="ExternalOutput").ap()

    with tile.TileContext(nc) as tc:
        _program(nc, tc, x_d, xtc_d, hid_d, w1t_d, w2t_d, b1s_d, b2s_d,
                 ident_d, g1b_d, g2b_d, bcb_d, out_d, act)

    nc.compile()
    return nc


def _program(nc, tc, x_d, xtc_d, hid_d, w1t_d, w2t_d, b1s_d, b2s_d, ident_d,
             g1b_d, g2b_d, bcb_d, out_d, act=AF.Gelu):
    with (
        tc.tile_pool(name="consts", bufs=1) as cpool,
        tc.tile_pool(name="hid", bufs=2) as hpool,
        tc.tile_pool(name="xt", bufs=2) as xtpool,
        tc.tile_pool(name="x", bufs=3) as xpool,
        tc.tile_pool(name="acc", bufs=2) as apool,
        tc.tile_pool(name="st", bufs=2) as stpool,
        tc.tile_pool(name="g", bufs=2) as gpool,
        tc.tile_pool(name="fg", bufs=2) as fgpool,
        tc.tile_pool(name="tmp", bufs=2) as tpool,
        tc.tile_pool(name="dum", bufs=1) as dpool,
        tc.tile_pool(name="stats", bufs=2) as spool,
        tc.tile_pool(name="ps1", bufs=2, space="PSUM") as ps1pool,
        tc.tile_pool(name="ps2", bufs=2, space="PSUM") as ps2pool,
        tc.tile_pool(name="pt", bufs=3, space="PSUM") as ptpool,
    ):
        # ---- small constants first (sync queue; ~50 KB total) ----
        b1s_sb = cpool.tile([P, MO], F32)
        nc.sync.dma_start(b1s_sb, b1s_d)
        b2s_sb = cpool.tile([1, H], BF16)
        nc.sync.dma_start(b2s_sb, b2s_d)
        ident_sb = cpool.tile([P, P], BF16)
        nc.sync.dma_start(ident_sb, ident_d)
        g1b_sb = cpool.tile([P, H], BF16)
        nc.sync.dma_start(g1b_sb, g1b_d)
        g2b_sb = cpool.tile([P, H], BF16)
        nc.sync.dma_start(g2b_sb, g2b_d)
        bcb_sb = cpool.tile([P, H], BF16)
        nc.sync.dma_start(bcb_sb, bcb_d)
        ones_sb = cpool.tile([1, P], BF16)
        nc.vector.memset(ones_sb, 1.0)

        w1t_sb = cpool.tile([P, KO1, H], BF16)
        w2t_sb = cpool.tile([P, KO2, H], BF16)
        w1t_r = w1t_d.rearrange("(ko p) n -> p ko n", p=P)

        dummy = dpool.tile([P, H], BF16, tag="dummy")

        for c, (t0, TC) in enumerate(CHUNKS):
            # ---- DMA issues (loads only; stores go at the chunk end) ----
            hA = hpool.tile([P, 13, H], BF16, tag="hid")
            nc.sync.dma_start(
                hA[0:TC],
                hid_d[0:13, t0:t0 + TC, :].rearrange("l p f -> p l f"))
            hB = hpool.tile([P, 13, H], BF16, tag="hid")
            nc.sync.dma_start(
                hB[0:TC],
                hid_d[13:26, t0:t0 + TC, :].rearrange("l p f -> p l f"))
            xt = xtpool.tile([P, MO, P], BF16, tag="xtc")
            nc.scalar.dma_start(xt, xtc_d[c])
            xc = xpool.tile([P, H], BF16, tag="x")
            nc.scalar.dma_start(xc[0:TC], x_d[t0:t0 + TC, :])
            if c == 0:
                # weights stream behind chunk 0 on the scalar queue
                for k0 in range(0, KO1, 9):
                    nc.scalar.dma_start(w1t_sb[:, k0:k0 + 9, :],
                                        w1t_r[:, k0:k0 + 9, :])
                nc.scalar.dma_start(
                    w2t_sb, w2t_d.rearrange("(ko p) n -> p ko n", p=P))

            # ---- 13-layer sums: s1 fully on DVE; s2 split GPSIMD/DVE ----
            s1 = apool.tile([P, H], BF16, tag="s1")
            nc.vector.tensor_add(s1[0:TC], hA[0:TC, 0, :], hA[0:TC, 1, :])
            for l in range(2, 13):
                nc.vector.tensor_add(s1[0:TC], s1[0:TC], hA[0:TC, l, :])
            s2a = apool.tile([P, H], BF16, tag="s2a")
            nc.gpsimd.tensor_add(s2a[0:TC], hB[0:TC, 0, :], hB[0:TC, 1, :])
            for l in range(2, 5):
                nc.gpsimd.tensor_add(s2a[0:TC], s2a[0:TC], hB[0:TC, l, :])
            s2 = apool.tile([P, H], BF16, tag="s2")
            nc.vector.tensor_add(s2[0:TC], hB[0:TC, 5, :], hB[0:TC, 6, :])
            for l in range(7, 13):
                nc.vector.tensor_add(s2[0:TC], s2[0:TC], hB[0:TC, l, :])
            nc.vector.tensor_add(s2[0:TC], s2[0:TC], s2a[0:TC])

            # ---- LN1(x) raw sums on ACT (accum_out) ----
            sacc = spool.tile([P, 8], F32, tag="sacc")
            nc.scalar.activation(dummy[0:TC], xc[0:TC], AF.Copy,
                                 accum_out=sacc[0:TC, 0:1])
            nc.scalar.activation(dummy[0:TC], xc[0:TC], AF.Square,
                                 accum_out=sacc[0:TC, 2:3])

            # ---- s1/s2 transposes: TensorE (identity) -> PSUM -> ACT ----
            sT = []
            for src, tg in ((s1, "s1T"), (s2, "s2T")):
                dst = stpool.tile([P, MO, P], BF16, tag=tg)
                for g0 in (0, 4, 8):
                    g = min(4, MO - g0)
                    pt = ptpool.tile([P, 4, P], BF16, tag="pt")
                    for j in range(g):
                        nc.tensor.transpose(
                            pt[:, j, 0:TC],
                            src[0:TC, (g0 + j) * P:(g0 + j + 1) * P],
                            ident_sb[0:TC, 0:TC])
                    nc.scalar.activation(dst[:, g0:g0 + g, 0:TC],
                                         pt[:, 0:g, 0:TC], AF.Copy)
                sT.append(dst)
            s1T, s2T = sT

            # ---- GEMM1 (weights-stationary) + GELU(+b1) -> g^T ----
            gT = gpool.tile([P, MO, P], BF16, tag="gT")
            for mm in range(MO):
                ps1 = ps1pool.tile([P, P], F32, tag="ps1")
                for kk in range(KO1):
                    if kk < MO:
                        rhs = xt[:, kk, 0:TC]
                    elif kk < 2 * MO:
                        rhs = s1T[:, kk - MO, 0:TC]
                    else:
                        rhs = s2T[:, kk - 2 * MO, 0:TC]
                    nc.tensor.matmul(
                        ps1[:, 0:TC],
                        lhsT=w1t_sb[:, kk, mm * P:(mm + 1) * P],
                        rhs=rhs,
                        start=(kk == 0),
                        stop=(kk == KO1 - 1),
                    )
                nc.scalar.activation(gT[:, mm, 0:TC], ps1[:, 0:TC], act,
                                     bias=b1s_sb[:, mm:mm + 1])

            # ---- GEMM2 (g^T-stationary) + b2 rank-1 + ACT drain/accum ----
            fg = fgpool.tile([P, H], BF16, tag="fg")
            for nn in range(NCHUNK):
                ps2 = ps2pool.tile([P, CH], F32, tag="ps2")
                for kk in range(KO2):
                    nc.tensor.matmul(
                        ps2[0:TC, :],
                        lhsT=gT[:, kk, 0:TC],
                        rhs=w2t_sb[:, kk, nn * CH:(nn + 1) * CH],
                        start=(kk == 0),
                        stop=False,
                    )
                nc.tensor.matmul(
                    ps2[0:TC, :],
                    lhsT=ones_sb[0:1, 0:TC],
                    rhs=b2s_sb[0:1, nn * CH:(nn + 1) * CH],
                    start=False,
                    stop=True,
                )
                nc.scalar.activation(fg[0:TC, nn * CH:(nn + 1) * CH],
                                     ps2[0:TC, :], AF.Copy,
                                     accum_out=sacc[0:TC, 4 + nn:5 + nn])
            nc.scalar.activation(dummy[0:TC], fg[0:TC], AF.Square,
                                 accum_out=sacc[0:TC, 3:4])

            # ---- LN stats -> mean / rsqrt(var+eps) for x and fg ----
            deriv = spool.tile([P, 8], F32, tag="deriv")
            nc.vector.tensor_add(sacc[0:TC, 1:2], sacc[0:TC, 4:5],
                                 sacc[0:TC, 5:6])
            nc.vector.tensor_add(sacc[0:TC, 1:2], sacc[0:TC, 1:2],
                                 sacc[0:TC, 6:7])
            # cols 0,1 = mean(x), mean(fg); 2,3 = E[v^2]+eps; 4,5 = mu^2
            nc.vector.tensor_scalar_mul(deriv[0:TC, 0:2], sacc[0:TC, 0:2],
                                        1.0 / H)
            nc.vector.tensor_scalar(deriv[0:TC, 2:4], sacc[0:TC, 2:4],
                                    1.0 / H, EPS, ALU.mult, ALU.add)
            nc.vector.tensor_tensor(deriv[0:TC, 4:6], deriv[0:TC, 0:2],
                                    deriv[0:TC, 0:2], ALU.mult)
            nc.vector.tensor_tensor(deriv[0:TC, 6:8], deriv[0:TC, 2:4],
                                    deriv[0:TC, 4:6], ALU.subtract)
            igt = spool.tile([P, 2], F32, tag="ig")
            nc.vector.reciprocal(igt[0:TC], deriv[0:TC, 6:8])
            nc.scalar.activation(igt[0:TC], igt[0:TC], AF.Sqrt)

            # ---- normalize + sigmoid gate, store ----
            tmp1 = tpool.tile([P, H], BF16, tag="tmp1")
            # tmp1 = (x - mu1) * G1;  G1 = (1-w)*ln1_g  (broadcast)
            nc.vector.scalar_tensor_tensor(
                tmp1[0:TC], xc[0:TC], deriv[0:TC, 0:1], g1b_sb[0:TC],
                ALU.subtract, ALU.mult)
            # fg <- (fg - mu2) * G2;  G2 = w*ln2_g   (in place)
            nc.vector.scalar_tensor_tensor(
                fg[0:TC], fg[0:TC], deriv[0:TC, 1:2], g2b_sb[0:TC],
                ALU.subtract, ALU.mult)
            # tmp1 = tmp1 * ig1 + Bc;  Bc = w*ln2_b + (1-w)*ln1_b
            nc.vector.scalar_tensor_tensor(
                tmp1[0:TC], tmp1[0:TC], igt[0:TC, 0:1], bcb_sb[0:TC],
                ALU.mult, ALU.add)
            # tmp1 <- fg * ig2 + tmp1   (final output)
            nc.vector.scalar_tensor_tensor(
                tmp1[0:TC], fg[0:TC], igt[0:TC, 1:2], tmp1[0:TC],
                ALU.mult, ALU.add)
            nc.scalar.dma_start(out_d[t0:t0 + TC, :], tmp1[0:TC])


_NC_CACHE = {}


def _get_nc():
    if "nc" not in _NC_CACHE:
        _NC_CACHE["nc"] = _build_program()
    return _NC_CACHE["nc"]


def _host_prep(codes1, scales1, b1, codes2, scales2, b2,
               ln1_g, ln1_b, ln2_g, ln2_b, alpha):
    # W1 with 1/13 folded into the f1/f2 column blocks (mean -> sum)
    w1 = _dequant_nf4(codes1, scales1)
    # match reference rounding: dequant result is cast to bf16 first
    w1 = w1.astype(NP_BF16).astype(np.float32)
    w1[:, H:] *= np.float32(1.0 / 13.0)
    w1t = np.ascontiguousarray(w1.T).astype(NP_BF16)

    w2 = _dequant_nf4(codes2, scales2).astype(NP_BF16)
    w2t = np.ascontiguousarray(w2.astype(np.float32).T).astype(NP_BF16)

    b1s = np.ascontiguousarray(
        b1.astype(np.float32).reshape(MO, P).T)  # [P, MO]
    b2s = np.ascontiguousarray(b2.astype(NP_BF16).reshape(1, H))

    ident = np.eye(P, dtype=NP_BF16)

    a32 = alpha.astype(np.float32)
    w_gate = (1.0 / (1.0 + np.exp(-a32[0]))).astype(NP_BF16)
    one_minus = (NP_BF16(1.0) - w_gate)
    g1 = (one_minus.astype(np.float32) * ln1_g.astype(np.float32))
    g2 = (w_gate.astype(np.float32) * ln2_g.astype(np.float32))
    bc = (w_gate.astype(np.float32) * ln2_b.astype(np.float32)
          + one_minus.astype(np.float32) * ln1_b.astype(np.float32))
    g1b = np.ascontiguousarray(np.broadcast_to(g1.astype(NP_BF16), (P, H)))
    g2b = np.ascontiguousarray(np.broadcast_to(g2.astype(NP_BF16), (P, H)))
    bcb = np.ascontiguousarray(np.broadcast_to(bc.astype(NP_BF16), (P, H)))
    return w1t, w2t, b1s, b2s, ident, g1b, g2b, bcb


def _xtc_prep(x):
    """[729, H] token-major -> [NCH, P, MO, P] feature-major token chunks."""
    xp = np.zeros((NTP, H), dtype=NP_BF16)
    xp[0:NT] = x
    arr = np.ascontiguousarray(
        xp.T.reshape(MO, P, NCH, P).transpose(2, 1, 0, 3))
    return arr


def make_in_maps(image_features, hidden, codes1, scales1, b1, codes2, scales2,
                 b2, ln1_g, ln1_b, ln2_g, ln2_b, alpha):
    w1t, w2t, b1s, b2s, ident, g1b, g2b, bcb = _host_prep(
        codes1, scales1, b1, codes2, scales2, b2,
        ln1_g, ln1_b, ln2_g, ln2_b, alpha)
    B = image_features.shape[0]
    in_maps = []
    for c in range(B):
        xc = np.ascontiguousarray(image_features[c]).astype(NP_BF16, copy=False)
        in_maps.append({
            "x": xc,
            "xtc": _xtc_prep(xc),
            "hid": np.ascontiguousarray(hidden[:, c]).astype(NP_BF16, copy=False),
            "w1t": w1t, "w2t": w2t, "b1s": b1s, "b2s": b2s, "ident": ident,
            "g1b": g1b, "g2b": g2b, "bcb": bcb,
        })
    return in_maps


def kernel(image_features, hidden, codes1, scales1, b1, codes2, scales2, b2,
           ln1_g, ln1_b, ln2_g, ln2_b, alpha, _trace=False):
    B, N, Hin = image_features.shape
    assert (B, N, Hin) == (8, NT, H), (B, N, Hin)
    nc = _get_nc()
    in_maps = make_in_maps(image_features, hidden, codes1, scales1, b1,
                           codes2, scales2, b2, ln1_g, ln1_b, ln2_g, ln2_b,
                           alpha)
    res = bass_utils.run_bass_kernel_spmd(
        nc, in_maps, core_ids=list(range(8)), trace=_trace)
    out = np.stack([res.results[c]["out"] for c in range(8)])
    if _trace:
        kernel._last_results = res
    return out.astype(image_features.dtype, copy=False)


# revision 4
# speedup vs baseline: 2.0548x; 2.0548x over previous
"""Trainium2 Bass kernel for nn_Connector_77738908057780 (dense_mlp).

Computation (see reference):
  x   = image_features                      [B, N, H]    bf16
  f1  = mean(hidden[0:13],  axis=0)         [B, N, H]
  f2  = mean(hidden[13:26], axis=0)         [B, N, H]
  cat = concat([x, f1, f2], -1)             [B, N, 3H]
  h   = gelu(cat @ W1.T + b1)               W1 = nf4_dequant(codes1, scales1) [H, 3H]
  fg  = h @ W2.T + b2                       W2 = nf4_dequant(codes2, scales2) [H, H]
  out = w * LN(fg) + (1-w) * LN(x),         w = sigmoid(alpha)

Sharding: data-parallel over batch B=8 -> one batch element per NeuronCore.

Per-core plan (v3 -- chunked pipeline):
  - 6 token chunks of 128 (last chunk overlaps the previous by 39 tokens;
    identical values stored twice -- partial-partition DMA falls off the
    16-engine SDMA path and runs ~15x slower, so all tiles stay full-128).
  - The 26-layer `hidden` stream dominates HBM traffic (46 MB/core); it is
    issued as 12 large 3.8 MB DMAs on the sync HWDGE queue in chunk order so
    DMA stays saturated end-to-end.  Weights stream on the scalar queue
    behind chunk 0's loads.
  - layer sums entirely on DVE (GPSIMD port contention halves DVE throughput
    when both run -- measured), tree-shaped to amortize dispatch and release
    the hid tile early for DMA slot reuse.
  - cat^T is never materialized: GEMM1's k-loop reads x^T (host-transposed
    input), s1^T and s2^T (TensorE identity-transpose -> PSUM -> ACT copy)
    as three separate SBUF tiles.  No SBUF->SBUF xbar DMA at all.
  - GEMM1 weights-stationary -> h^T in PSUM; GELU(+b1 per-partition bias) on
    ACT -> g^T feeds GEMM2 as stationary; b2 is added by a rank-1 matmul
    (ones-row x b2-row) inside the accumulation group; ACT drains PSUM->fg
    while computing sum(fg) via accum_out.
  - LN stats: ACT accum_out gives S(v), S(v^2); DVE combines to mean/var,
    reciprocal+sqrt for rsqrt.  The gate combine uses 4x-mode tensor_scalar
    ops when the folded LN gains are feature-uniform (they are: ln gains are
    ones, biases zeros), falling back to scalar_tensor_tensor otherwise.

NF4 dequant of the (small, replicated) weights is host-side weight prep; the
bf16 weights are less DMA traffic than the int32 codes.
"""

import os
import sys

import numpy as np
import ml_dtypes

for _p in ("/opt/trn_rl_repo", "/root/.axon_site/_ro/trn_rl_repo"):
    if os.path.isdir(_p) and _p not in sys.path:
        sys.path.insert(0, _p)

import concourse.bass as bass
import concourse.mybir as mybir
import concourse.tile as tile
from concourse import bacc
from concourse import bass_utils

BF16 = mybir.dt.bfloat16
F32 = mybir.dt.float32
AF = mybir.ActivationFunctionType
ALU = mybir.AluOpType

NP_BF16 = ml_dtypes.bfloat16

P = 128
H = 1152
H3 = 3456
NT = 729          # tokens per core (N); B=8 cores
L = 26
KO1 = H3 // P     # 27 k-tiles for GEMM1
KO2 = H // P      # 9 k-tiles for GEMM2
MO = H // P       # 9 output-feature tiles
EPS = 1e-5
NCHUNK = 3        # fg free-dim chunks of 384
CH = H // NCHUNK  # 384

# Token chunks; the last starts at 601 so it is a full 128 tokens (tokens
# 601..639 are computed twice with identical values).
CHUNK_STARTS = [0, 128, 256, 384, 512, 601]
NCH = len(CHUNK_STARTS)

NF4_CODEBOOK = np.array([
    -1.0, -0.6961928009986877, -0.5250730514526367, -0.39491748809814453,
    -0.28444138169288635, -0.18477343022823334, -0.09105003625154495, 0.0,
    0.07958029955625534, 0.16093020141124725, 0.24611230194568634,
    0.33791524171829224, 0.4407098591327667, 0.5626170039176941,
    0.7229568362236023, 1.0], dtype=np.float32)

BLOCK = 64


def _dequant_nf4(codes, scales):
    """Match reference: codebook lookup * per-64-block absmax, cast bf16."""
    out_f, in_f = codes.shape
    w = NF4_CODEBOOK[codes].reshape(out_f, in_f // BLOCK, BLOCK)
    w = w * scales[:, :, None].astype(np.float32)
    return w.reshape(out_f, in_f)  # float32 (caller casts)


def _build_program(act=AF.Gelu, uniform_gate=True):
    nc = bacc.Bacc(
        "TRN2",
        target_bir_lowering=False,
        debug=False,
        num_devices=1,
    )
    x_d = nc.dram_tensor("x", (NT, H), BF16, kind="ExternalInput").ap()
    xtc_d = nc.dram_tensor("xtc", (NCH, P, MO, P), BF16, kind="ExternalInput").ap()
    hid_d = nc.dram_tensor("hid", (L, NT, H), BF16, kind="ExternalInput").ap()
    w1t_d = nc.dram_tensor("w1t", (H3, H), BF16, kind="ExternalInput").ap()
    w2t_d = nc.dram_tensor("w2t", (H, H), BF16, kind="ExternalInput").ap()
    b1s_d = nc.dram_tensor("b1s", (P, MO), F32, kind="ExternalInput").ap()
    b2s_d = nc.dram_tensor("b2s", (1, H), BF16, kind="ExternalInput").ap()
    ident_d = nc.dram_tensor("ident", (P, P), BF16, kind="ExternalInput").ap()
    # uniform path: gsc = per-partition (G1, G2) scalars; bcs = (Bc, 0)
    gsc_d = nc.dram_tensor("gsc", (P, 2), F32, kind="ExternalInput").ap()
    bcs_d = nc.dram_tensor("bcs", (P, 2), F32, kind="ExternalInput").ap()
    # general path: per-feature broadcasts
    g1b_d = nc.dram_tensor("g1b", (P, H), BF16, kind="ExternalInput").ap()
    g2b_d = nc.dram_tensor("g2b", (P, H), BF16, kind="ExternalInput").ap()
    bcb_d = nc.dram_tensor("bcb", (P, H), BF16, kind="ExternalInput").ap()
    out_d = nc.dram_tensor("out", (NT, H), BF16, kind="ExternalOutput").ap()

    with tile.TileContext(nc) as tc:
        _program(nc, tc, x_d, xtc_d, hid_d, w1t_d, w2t_d, b1s_d, b2s_d,
                 ident_d, gsc_d, bcs_d, g1b_d, g2b_d, bcb_d, out_d, act,
                 uniform_gate)

    nc.compile()
    return nc


def _program(nc, tc, x_d, xtc_d, hid_d, w1t_d, w2t_d, b1s_d, b2s_d, ident_d,
             gsc_d, bcs_d, g1b_d, g2b_d, bcb_d, out_d, act, uniform_gate):
    with (
        tc.tile_pool(name="consts", bufs=1) as cpool,
        tc.tile_pool(name="hid", bufs=2) as hpool,
        tc.tile_pool(name="xt", bufs=2) as xtpool,
        tc.tile_pool(name="x", bufs=3) as xpool,
        tc.tile_pool(name="scr", bufs=1) as scrpool,
        tc.tile_pool(name="acc", bufs=2) as apool,
        tc.tile_pool(name="st", bufs=2) as stpool,
        tc.tile_pool(name="g", bufs=2) as gpool,
        tc.tile_pool(name="fg", bufs=2) as fgpool,
        tc.tile_pool(name="tmp", bufs=2) as tpool,
        tc.tile_pool(name="dum", bufs=1) as dpool,
        tc.tile_pool(name="stats", bufs=2) as spool,
        tc.tile_pool(name="ps1", bufs=2, space="PSUM") as ps1pool,
        tc.tile_pool(name="ps2", bufs=2, space="PSUM") as ps2pool,
        tc.tile_pool(name="pt", bufs=3, space="PSUM") as ptpool,
    ):
        # ---- small constants first (sync queue; ~50 KB total) ----
        b1s_sb = cpool.tile([P, MO], F32)
        nc.sync.dma_start(b1s_sb, b1s_d)
        b2s_sb = cpool.tile([1, H], BF16)
        nc.sync.dma_start(b2s_sb, b2s_d)
        ident_sb = cpool.tile([P, P], BF16)
        nc.sync.dma_start(ident_sb, ident_d)
        if uniform_gate:
            gsc_sb = cpool.tile([P, 2], F32)
            nc.sync.dma_start(gsc_sb, gsc_d)
            bcs_sb = cpool.tile([P, 2], F32)
            nc.sync.dma_start(bcs_sb, bcs_d)
        else:
            g1b_sb = cpool.tile([P, H], BF16)
            nc.sync.dma_start(g1b_sb, g1b_d)
            g2b_sb = cpool.tile([P, H], BF16)
            nc.sync.dma_start(g2b_sb, g2b_d)
            bcb_sb = cpool.tile([P, H], BF16)
            nc.sync.dma_start(bcb_sb, bcb_d)
        ones_sb = cpool.tile([1, P], BF16)
        nc.vector.memset(ones_sb, 1.0)

        w1t_sb = cpool.tile([P, KO1, H], BF16)
        w2t_sb = cpool.tile([P, KO2, H], BF16)
        w1t_r = w1t_d.rearrange("(ko p) n -> p ko n", p=P)

        dummy = dpool.tile([P, H], BF16, tag="dummy")
        scratch = scrpool.tile([P, 6, H], BF16, tag="scr")

        def half_sum(hT, lo, dst):
            """dst[t, f] = sum_l hT[t, lo+l, f] for l in 0..12, on DVE.

            Tree-shaped: the big first level amortizes DVE dispatch, and hT
            is fully consumed after the second op so its DMA slot recycles
            early."""
            a = scratch
            nc.vector.tensor_add(a, hT[:, lo:lo + 6, :], hT[:, lo + 6:lo + 12, :])
            nc.vector.tensor_add(a[:, 5, :], a[:, 5, :], hT[:, lo + 12, :])
            nc.vector.tensor_add(a[:, 0:3, :], a[:, 0:3, :], a[:, 3:6, :])
            nc.vector.tensor_add(dst, a[:, 0, :], a[:, 1, :])
            nc.vector.tensor_add(dst, dst, a[:, 2, :])

        for c, t0 in enumerate(CHUNK_STARTS):
            # ---- DMA issues (loads only; stores go at the chunk end) ----
            hA = hpool.tile([P, 13, H], BF16, tag="hid")
            nc.sync.dma_start(
                hA, hid_d[0:13, t0:t0 + P, :].rearrange("l p f -> p l f"))
            hB = hpool.tile([P, 13, H], BF16, tag="hid")
            nc.sync.dma_start(
                hB, hid_d[13:26, t0:t0 + P, :].rearrange("l p f -> p l f"))
            xt = xtpool.tile([P, MO, P], BF16, tag="xtc")
            nc.scalar.dma_start(xt, xtc_d[c])
            xc = xpool.tile([P, H], BF16, tag="x")
            nc.scalar.dma_start(xc, x_d[t0:t0 + P, :])
            if c == 0:
                # weights stream behind chunk 0 on the scalar queue
                for k0 in range(0, KO1, 9):
                    nc.scalar.dma_start(w1t_sb[:, k0:k0 + 9, :],
                                        w1t_r[:, k0:k0 + 9, :])
                nc.scalar.dma_start(
                    w2t_sb, w2t_d.rearrange("(ko p) n -> p ko n", p=P))

            # ---- 13-layer sums on DVE ----
            s1 = apool.tile([P, H], BF16, tag="s1")
            half_sum(hA, 0, s1)
            s2 = apool.tile([P, H], BF16, tag="s2")
            half_sum(hB, 0, s2)

            # ---- LN1(x) raw sums on ACT (accum_out) ----
            sacc = spool.tile([P, 8], F32, tag="sacc")
            nc.scalar.activation(dummy, xc, AF.Copy,
                                 accum_out=sacc[:, 0:1])
            nc.scalar.activation(dummy, xc, AF.Square,
                                 accum_out=sacc[:, 2:3])

            # ---- s1/s2 transposes: TensorE (identity) -> PSUM -> ACT ----
            sT = []
            for src, tg in ((s1, "s1T"), (s2, "s2T")):
                dst = stpool.tile([P, MO, P], BF16, tag=tg)
                for g0 in (0, 4, 8):
                    g = min(4, MO - g0)
                    pt = ptpool.tile([P, 4, P], BF16, tag="pt")
                    for j in range(g):
                        nc.tensor.transpose(
                            pt[:, j, :],
                            src[:, (g0 + j) * P:(g0 + j + 1) * P],
                            ident_sb)
                    nc.scalar.activation(dst[:, g0:g0 + g, :],
                                         pt[:, 0:g, :], AF.Copy)
                sT.append(dst)
            s1T, s2T = sT

            # ---- GEMM1 (weights-stationary) + GELU(+b1) -> g^T ----
            gT = gpool.tile([P, MO, P], BF16, tag="gT")
            for mm in range(MO):
                ps1 = ps1pool.tile([P, P], F32, tag="ps1")
                for kk in range(KO1):
                    if kk < MO:
                        rhs = xt[:, kk, :]
                    elif kk < 2 * MO:
                        rhs = s1T[:, kk - MO, :]
                    else:
                        rhs = s2T[:, kk - 2 * MO, :]
                    nc.tensor.matmul(
                        ps1,
                        lhsT=w1t_sb[:, kk, mm * P:(mm + 1) * P],
                        rhs=rhs,
                        start=(kk == 0),
                        stop=(kk == KO1 - 1),
                    )
                nc.scalar.activation(gT[:, mm, :], ps1, act,
                                     bias=b1s_sb[:, mm:mm + 1])

            # ---- GEMM2 (g^T-stationary) + b2 rank-1 + ACT drain/accum ----
            fg = fgpool.tile([P, H], BF16, tag="fg")
            for nn in range(NCHUNK):
                ps2 = ps2pool.tile([P, CH], F32, tag="ps2")
                for kk in range(KO2):
                    nc.tensor.matmul(
                        ps2,
                        lhsT=gT[:, kk, :],
                        rhs=w2t_sb[:, kk, nn * CH:(nn + 1) * CH],
                        start=(kk == 0),
                        stop=False,
                    )
                nc.tensor.matmul(
                    ps2,
                    lhsT=ones_sb,
                    rhs=b2s_sb[0:1, nn * CH:(nn + 1) * CH],
                    start=False,
                    stop=True,
                )
                nc.scalar.activation(fg[:, nn * CH:(nn + 1) * CH],
                                     ps2, AF.Copy,
                                     accum_out=sacc[:, 4 + nn:5 + nn])
            nc.scalar.activation(dummy, fg, AF.Square,
                                 accum_out=sacc[:, 3:4])

            # ---- LN stats -> mean / rsqrt(var+eps) for x and fg ----
            deriv = spool.tile([P, 8], F32, tag="deriv")
            nc.vector.tensor_add(sacc[:, 1:2], sacc[:, 4:5], sacc[:, 5:6])
            nc.vector.tensor_add(sacc[:, 1:2], sacc[:, 1:2], sacc[:, 6:7])
            # cols 0,1 = mean(x), mean(fg); 2,3 = E[v^2]+eps; 4,5 = mu^2
            nc.vector.tensor_scalar_mul(deriv[:, 0:2], sacc[:, 0:2], 1.0 / H)
            nc.vector.tensor_scalar(deriv[:, 2:4], sacc[:, 2:4],
                                    1.0 / H, EPS, ALU.mult, ALU.add)
            nc.vector.tensor_tensor(deriv[:, 4:6], deriv[:, 0:2],
                                    deriv[:, 0:2], ALU.mult)
            nc.vector.tensor_tensor(deriv[:, 6:8], deriv[:, 2:4],
                                    deriv[:, 4:6], ALU.subtract)
            igt = spool.tile([P, 2], F32, tag="ig")
            nc.vector.reciprocal(igt, deriv[:, 6:8])
            nc.scalar.activation(igt, igt, AF.Sqrt)

            # ---- normalize + sigmoid gate, store ----
            tmp1 = tpool.tile([P, H], BF16, tag="tmp1")
            if uniform_gate:
                # acol = (G1*ig1, G2*ig2); Bc folded via bcs (always 0 here)
                acol = spool.tile([P, 2], F32, tag="acol")
                nc.vector.tensor_tensor(acol, igt, gsc_sb, ALU.mult)
                # tmp1 = (x - mu1) * a1   (4x-mode tensor_scalar)
                nc.vector.tensor_scalar(tmp1, xc, deriv[:, 0:1],
                                        acol[:, 0:1], ALU.subtract, ALU.mult)
                # fg <- (fg - mu2) * a2   (in place)
                nc.vector.tensor_scalar(fg, fg, deriv[:, 1:2],
                                        acol[:, 1:2], ALU.subtract, ALU.mult)
                # tmp1 <- (tmp1 + Bc) + fg
                nc.vector.scalar_tensor_tensor(
                    tmp1, tmp1, bcs_sb[:, 0:1], fg, ALU.add, ALU.add)
            else:
                # tmp1 = (x - mu1) * G1;  G1 = (1-w)*ln1_g  (broadcast)
                nc.vector.scalar_tensor_tensor(
                    tmp1, xc, deriv[:, 0:1], g1b_sb,
                    ALU.subtract, ALU.mult)
                # fg <- (fg - mu2) * G2;  G2 = w*ln2_g   (in place)
                nc.vector.scalar_tensor_tensor(
                    fg, fg, deriv[:, 1:2], g2b_sb,
                    ALU.subtract, ALU.mult)
                # tmp1 = tmp1 * ig1 + Bc;  Bc = w*ln2_b + (1-w)*ln1_b
                nc.vector.scalar_tensor_tensor(
                    tmp1, tmp1, igt[:, 0:1], bcb_sb,
                    ALU.mult, ALU.add)
                # tmp1 <- fg * ig2 + tmp1   (final output)
                nc.vector.scalar_tensor_tensor(
                    tmp1, fg, igt[:, 1:2], tmp1,
                    ALU.mult, ALU.add)
            nc.scalar.dma_start(out_d[t0:t0 + P, :], tmp1)


_NC_CACHE = {}


def _get_nc(uniform_gate=True):
    key = ("nc", uniform_gate)
    if key not in _NC_CACHE:
        _NC_CACHE[key] = _build_program(uniform_gate=uniform_gate)
    return _NC_CACHE[key]


def _host_prep(codes1, scales1, b1, codes2, scales2, b2,
               ln1_g, ln1_b, ln2_g, ln2_b, alpha):
    # W1 with 1/13 folded into the f1/f2 column blocks (mean -> sum)
    w1 = _dequant_nf4(codes1, scales1)
    # match reference rounding: dequant result is cast to bf16 first
    w1 = w1.astype(NP_BF16).astype(np.float32)
    w1[:, H:] *= np.float32(1.0 / 13.0)
    w1t = np.ascontiguousarray(w1.T).astype(NP_BF16)

    w2 = _dequant_nf4(codes2, scales2).astype(NP_BF16)
    w2t = np.ascontiguousarray(w2.astype(np.float32).T).astype(NP_BF16)

    b1s = np.ascontiguousarray(
        b1.astype(np.float32).reshape(MO, P).T)  # [P, MO]
    b2s = np.ascontiguousarray(b2.astype(NP_BF16).reshape(1, H))

    ident = np.eye(P, dtype=NP_BF16)

    a32 = alpha.astype(np.float32)
    w_gate = (1.0 / (1.0 + np.exp(-a32[0]))).astype(NP_BF16)
    one_minus = (NP_BF16(1.0) - w_gate)
    g1 = (one_minus.astype(np.float32) * ln1_g.astype(np.float32))
    g2 = (w_gate.astype(np.float32) * ln2_g.astype(np.float32))
    bc = (w_gate.astype(np.float32) * ln2_b.astype(np.float32)
          + one_minus.astype(np.float32) * ln1_b.astype(np.float32))

    uniform = (np.ptp(g1) == 0.0 and np.ptp(g2) == 0.0 and np.all(bc == 0.0))
    gsc = np.ascontiguousarray(
        np.broadcast_to(np.array([g1[0], g2[0]], np.float32), (P, 2)))
    bcs = np.zeros((P, 2), np.float32)

    g1b = np.ascontiguousarray(np.broadcast_to(g1.astype(NP_BF16), (P, H)))
    g2b = np.ascontiguousarray(np.broadcast_to(g2.astype(NP_BF16), (P, H)))
    bcb = np.ascontiguousarray(np.broadcast_to(bc.astype(NP_BF16), (P, H)))
    return w1t, w2t, b1s, b2s, ident, gsc, bcs, g1b, g2b, bcb, uniform


def _xtc_prep(x):
    """[729, H] token-major -> [NCH, P, MO, P] feature-major token chunks."""
    xT = np.ascontiguousarray(x.T).reshape(MO, P, NT)
    out = np.empty((NCH, P, MO, P), dtype=NP_BF16)
    for c, t0 in enumerate(CHUNK_STARTS):
        out[c] = xT[:, :, t0:t0 + P].transpose(1, 0, 2)
    return out


def make_in_maps(image_features, hidden, codes1, scales1, b1, codes2, scales2,
                 b2, ln1_g, ln1_b, ln2_g, ln2_b, alpha):
    (w1t, w2t, b1s, b2s, ident, gsc, bcs, g1b, g2b, bcb,
     uniform) = _host_prep(codes1, scales1, b1, codes2, scales2, b2,
                           ln1_g, ln1_b, ln2_g, ln2_b, alpha)
    B = image_features.shape[0]
    in_maps = []
    for c in range(B):
        xc = np.ascontiguousarray(image_features[c]).astype(NP_BF16, copy=False)
        in_maps.append({
            "x": xc,
            "xtc": _xtc_prep(xc),
            "hid": np.ascontiguousarray(hidden[:, c]).astype(NP_BF16, copy=False),
            "w1t": w1t, "w2t": w2t, "b1s": b1s, "b2s": b2s, "ident": ident,
            "gsc": gsc, "bcs": bcs,
            "g1b": g1b, "g2b": g2b, "bcb": bcb,
        })
    return in_maps, uniform


def kernel(image_features, hidden, codes1, scales1, b1, codes2, scales2, b2,
           ln1_g, ln1_b, ln2_g, ln2_b, alpha, _trace=False):
    B, N, Hin = image_features.shape
    assert (B, N, Hin) == (8, NT, H), (B, N, Hin)
    in_maps, uniform = make_in_maps(
        image_features, hidden, codes1, scales1, b1, codes2, scales2, b2,
        ln1_g, ln1_b, ln2_g, ln2_b, alpha)
    nc = _get_nc(uniform_gate=uniform)
    res = bass_utils.run_bass_kernel_spmd(
        nc, in_maps, core_ids=list(range(8)), trace=_trace)
    out = np.stack([res.results[c]["out"] for c in range(8)])
    if _trace:
        kernel._last_results = res
    return out.astype(image_features.dtype, copy=False)


# revision 12
# speedup vs baseline: 2.1785x; 1.0602x over previous
"""Trainium2 Bass kernel for nn_Connector_77738908057780 (dense_mlp).

Computation (see reference):
  x   = image_features                      [B, N, H]    bf16
  f1  = mean(hidden[0:13],  axis=0)         [B, N, H]
  f2  = mean(hidden[13:26], axis=0)         [B, N, H]
  cat = concat([x, f1, f2], -1)             [B, N, 3H]
  h   = gelu(cat @ W1.T + b1)               W1 = nf4_dequant(codes1, scales1) [H, 3H]
  fg  = h @ W2.T + b2                       W2 = nf4_dequant(codes2, scales2) [H, H]
  out = w * LN(fg) + (1-w) * LN(x),         w = sigmoid(alpha)

Sharding: data-parallel over batch B=8 -> one batch element per NeuronCore.

Per-core plan (v3 -- chunked pipeline):
  - 6 token chunks of 128 (last chunk overlaps the previous by 39 tokens;
    identical values stored twice -- partial-partition DMA falls off the
    16-engine SDMA path and runs ~15x slower, so all tiles stay full-128).
  - The 26-layer `hidden` stream dominates HBM traffic (46 MB/core); it is
    issued as 12 large 3.8 MB DMAs on the sync HWDGE queue in chunk order so
    DMA stays saturated end-to-end.  Weights stream on the scalar queue
    behind chunk 0's loads.
  - layer sums entirely on DVE (GPSIMD port contention halves DVE throughput
    when both run -- measured), tree-shaped to amortize dispatch and release
    the hid tile early for DMA slot reuse.
  - cat^T is never materialized: GEMM1's k-loop reads x^T (host-transposed
    input), s1^T and s2^T (TensorE identity-transpose -> PSUM -> ACT copy)
    as three separate SBUF tiles.  No SBUF->SBUF xbar DMA at all.
  - GEMM1 weights-stationary -> h^T in PSUM; GELU(+b1 per-partition bias) on
    ACT -> g^T feeds GEMM2 as stationary; b2 is added by a rank-1 matmul
    (ones-row x b2-row) inside the accumulation group; ACT drains PSUM->fg
    while computing sum(fg) via accum_out.
  - LN stats: ACT accum_out gives S(v), S(v^2); DVE combines to mean/var,
    reciprocal+sqrt for rsqrt.  The gate combine uses 4x-mode tensor_scalar
    ops when the folded LN gains are feature-uniform (they are: ln gains are
    ones, biases zeros), falling back to scalar_tensor_tensor otherwise.

NF4 dequant of the (small, replicated) weights is host-side weight prep; the
bf16 weights are less DMA traffic than the int32 codes.
"""

import os
import sys

import numpy as np
import ml_dtypes

for _p in ("/opt/trn_rl_repo", "/root/.axon_site/_ro/trn_rl_repo"):
    if os.path.isdir(_p) and _p not in sys.path:
        sys.path.insert(0, _p)

import concourse.bass as bass
import concourse.mybir as mybir
import concourse.tile as tile
from concourse import bacc
from concourse import bass_utils

BF16 = mybir.dt.bfloat16
F32 = mybir.dt.float32
AF = mybir.ActivationFunctionType
ALU = mybir.AluOpType

NP_BF16 = ml_dtypes.bfloat16

P = 128
H = 1152
H3 = 3456
NT = 729          # tokens per core (N); B=8 cores
L = 26
KO1 = H3 // P     # 27 k-tiles for GEMM1
KO2 = H // P      # 9 k-tiles for GEMM2
MO = H // P       # 9 output-feature tiles
EPS = 1e-5
NCHUNK = 3        # fg free-dim chunks of 384
CH = H // NCHUNK  # 384

# Token chunks; the last starts at 601 so it is a full 128 tokens (tokens
# 601..639 are computed twice with identical values).
CHUNK_STARTS = [0, 128, 256, 384, 512, 601]
NCH = len(CHUNK_STARTS)

NF4_CODEBOOK = np.array([
    -1.0, -0.6961928009986877, -0.5250730514526367, -0.39491748809814453,
    -0.28444138169288635, -0.18477343022823334, -0.09105003625154495, 0.0,
    0.07958029955625534, 0.16093020141124725, 0.24611230194568634,
    0.33791524171829224, 0.4407098591327667, 0.5626170039176941,
    0.7229568362236023, 1.0], dtype=np.float32)

BLOCK = 64


def _dequant_nf4(codes, scales):
    """Match reference: codebook lookup * per-64-block absmax, cast bf16."""
    out_f, in_f = codes.shape
    w = NF4_CODEBOOK[codes].reshape(out_f, in_f // BLOCK, BLOCK)
    w = w * scales[:, :, None].astype(np.float32)
    return w.reshape(out_f, in_f)  # float32 (caller casts)


def _build_program(act=AF.Gelu, uniform_gate=True):
    nc = bacc.Bacc(
        "TRN2",
        target_bir_lowering=False,
        debug=False,
        num_devices=1,
    )
    x_d = nc.dram_tensor("x", (NT, H), BF16, kind="ExternalInput").ap()
    xtc_d = nc.dram_tensor("xtc", (NCH, P, MO, P), BF16, kind="ExternalInput").ap()
    hid_d = nc.dram_tensor("hid", (L, NT, H), BF16, kind="ExternalInput").ap()
    w1t_d = nc.dram_tensor("w1t", (H3, H), BF16, kind="ExternalInput").ap()
    w2t_d = nc.dram_tensor("w2t", (H, H), BF16, kind="ExternalInput").ap()
    b1s_d = nc.dram_tensor("b1s", (P, MO), F32, kind="ExternalInput").ap()
    b2s_d = nc.dram_tensor("b2s", (1, H), BF16, kind="ExternalInput").ap()
    ident_d = nc.dram_tensor("ident", (P, P), BF16, kind="ExternalInput").ap()
    # uniform path: gsc = per-partition (G1, G2) scalars; bcs = (Bc, 0)
    gsc_d = nc.dram_tensor("gsc", (P, 2), F32, kind="ExternalInput").ap()
    bcs_d = nc.dram_tensor("bcs", (P, 2), F32, kind="ExternalInput").ap()
    # general path: per-feature broadcasts
    g1b_d = nc.dram_tensor("g1b", (P, H), BF16, kind="ExternalInput").ap()
    g2b_d = nc.dram_tensor("g2b", (P, H), BF16, kind="ExternalInput").ap()
    bcb_d = nc.dram_tensor("bcb", (P, H), BF16, kind="ExternalInput").ap()
    out_d = nc.dram_tensor("out", (NT, H), BF16, kind="ExternalOutput").ap()

    with tile.TileContext(nc) as tc:
        _program(nc, tc, x_d, xtc_d, hid_d, w1t_d, w2t_d, b1s_d, b2s_d,
                 ident_d, gsc_d, bcs_d, g1b_d, g2b_d, bcb_d, out_d, act,
                 uniform_gate)

    nc.compile()
    return nc


def _program(nc, tc, x_d, xtc_d, hid_d, w1t_d, w2t_d, b1s_d, b2s_d, ident_d,
             gsc_d, bcs_d, g1b_d, g2b_d, bcb_d, out_d, act, uniform_gate):
    with (
        tc.tile_pool(name="consts", bufs=1) as cpool,
        tc.tile_pool(name="hid", bufs=4) as hpool,
        tc.tile_pool(name="xt", bufs=2) as xtpool,
        tc.tile_pool(name="x", bufs=3) as xpool,
        tc.tile_pool(name="scr", bufs=1) as scrpool,
        tc.tile_pool(name="acc", bufs=2) as apool,
        tc.tile_pool(name="st", bufs=2) as stpool,
        tc.tile_pool(name="g", bufs=2) as gpool,
        tc.tile_pool(name="fg", bufs=2) as fgpool,
        tc.tile_pool(name="tmp", bufs=2) as tpool,
        tc.tile_pool(name="dum", bufs=1) as dpool,
        tc.tile_pool(name="stats", bufs=2) as spool,
        tc.tile_pool(name="ps1", bufs=1, space="PSUM") as ps1pool,
        tc.tile_pool(name="ps2", bufs=2, space="PSUM") as ps2pool,
        tc.tile_pool(name="pt", bufs=3, space="PSUM") as ptpool,
    ):
        # ---- small constants first (sync queue; ~50 KB total) ----
        b1s_sb = cpool.tile([P, MO], F32)
        nc.sync.dma_start(b1s_sb, b1s_d)
        b2s_sb = cpool.tile([1, H], BF16)
        nc.sync.dma_start(b2s_sb, b2s_d)
        ident_sb = cpool.tile([P, P], BF16)
        nc.sync.dma_start(ident_sb, ident_d)
        if uniform_gate:
            gsc_sb = cpool.tile([P, 2], F32)
            nc.sync.dma_start(gsc_sb, gsc_d)
            bcs_sb = cpool.tile([P, 2], F32)
            nc.sync.dma_start(bcs_sb, bcs_d)
        else:
            g1b_sb = cpool.tile([P, H], BF16)
            nc.sync.dma_start(g1b_sb, g1b_d)
            g2b_sb = cpool.tile([P, H], BF16)
            nc.sync.dma_start(g2b_sb, g2b_d)
            bcb_sb = cpool.tile([P, H], BF16)
            nc.sync.dma_start(bcb_sb, bcb_d)
        ones_sb = cpool.tile([1, P], BF16)
        nc.vector.memset(ones_sb, 1.0)

        w1t_sb = cpool.tile([P, KO1, H], BF16)
        w2t_sb = cpool.tile([P, KO2, H], BF16)
        w1t_r = w1t_d.rearrange("(ko p) n -> p ko n", p=P)

        dummy = dpool.tile([P, H], BF16, tag="dummy")
        # DVE-serial scratch for the layer-sum trees (reused across chunks)
        scr = [scrpool.tile([P, 3, H], BF16, name=f"scr{i}", tag=f"scr{i}")
               for i in range(2)]

        def half_sum(h7, h6, dst, scr):
            """dst[t, f] = sum over the 7-layer and 6-layer pieces, on DVE.

            Tree-shaped to amortize DVE dispatch; each hid piece is fully
            consumed after two ops so its DMA slot recycles early."""
            t7 = scr[0]
            nc.vector.tensor_add(t7, h7[:, 0:3, :], h7[:, 3:6, :])
            nc.vector.tensor_add(t7[:, 2, :], t7[:, 2, :], h7[:, 6, :])
            t6 = scr[1]
            nc.vector.tensor_add(t6, h6[:, 0:3, :], h6[:, 3:6, :])
            nc.vector.tensor_add(t7, t7, t6)
            nc.vector.tensor_add(dst, t7[:, 0, :], t7[:, 1, :])
            nc.vector.tensor_add(dst, dst, t7[:, 2, :])

        def transpose_to(src, dst):
            """src [P, H] token-major -> dst [P, MO, P] feature-major."""
            for g0 in (0, 4, 8):
                g = min(4, MO - g0)
                pt = ptpool.tile([P, 4, P], BF16, tag="pt")
                for j in range(g):
                    nc.tensor.transpose(
                        pt[:, j, :],
                        src[:, (g0 + j) * P:(g0 + j + 1) * P],
                        ident_sb)
                nc.scalar.activation(dst[:, g0:g0 + g, :],
                                     pt[:, 0:g, :], AF.Copy)

        for c, t0 in enumerate(CHUNK_STARTS):
            # ---- DMA issues (loads only; stores go at the chunk end) ----
            hps = []
            for l0, nl in ((0, 7), (7, 6), (13, 7), (20, 6)):
                hp = hpool.tile([P, 7, H], BF16, tag="hid")
                nc.sync.dma_start(
                    hp[:, 0:nl, :],
                    hid_d[l0:l0 + nl, t0:t0 + P, :].rearrange(
                        "l p f -> p l f"))
                hps.append(hp)
            xt = xtpool.tile([P, MO, P], BF16, tag="xtc")
            nc.scalar.dma_start(xt, xtc_d[c])
            if c == 0:
                # weights stream behind chunk 0's x^T on the scalar queue,
                # ordered so GEMM1's k-outer loop can start early
                nc.scalar.dma_start(w1t_sb[:, 0:9, :], w1t_r[:, 0:9, :])
            xc = xpool.tile([P, H], BF16, tag="x")
            nc.scalar.dma_start(xc, x_d[t0:t0 + P, :])
            if c == 0:
                nc.scalar.dma_start(w1t_sb[:, 9:18, :], w1t_r[:, 9:18, :])
                nc.scalar.dma_start(w1t_sb[:, 18:27, :], w1t_r[:, 18:27, :])
                nc.scalar.dma_start(
                    w2t_sb, w2t_d.rearrange("(ko p) n -> p ko n", p=P))

            # ---- 13-layer sums on DVE ----
            s1 = apool.tile([P, H], BF16, tag="s1")
            half_sum(hps[0], hps[1], s1, scr)
            s2 = apool.tile([P, H], BF16, tag="s2")
            half_sum(hps[2], hps[3], s2, scr)

            # ---- LN1(x) raw sums on ACT (accum_out) ----
            sacc = spool.tile([P, 8], F32, tag="sacc")
            nc.scalar.activation(dummy, xc, AF.Copy,
                                 accum_out=sacc[:, 0:1])
            nc.scalar.activation(dummy, xc, AF.Square,
                                 accum_out=sacc[:, 2:3])

            # ---- GEMM1 (weights-stationary, k-outer) + interleaved s1/s2
            # transposes (TensorE identity -> PSUM -> ACT), then GELU -> g^T
            ps1 = ps1pool.tile([P, MO, P], F32, tag="ps1")
            s1T = stpool.tile([P, MO, P], BF16, tag="s1T")
            s2T = stpool.tile([P, MO, P], BF16, tag="s2T")

            def k_group(klo, khi, rhs_of):
                for kk in range(klo, khi):
                    rhs = rhs_of(kk)
                    for mm in range(MO):
                        # start=True marks the whole 2KB PSUM bank pending-
                        # zero, so only the first matmul touching each bank
                        # sets it; the other m-slices' first writes land on
                        # still-pending bytes and overwrite (HW has_written
                        # semantics, mirrored by the sim).
                        nc.tensor.matmul(
                            ps1[:, mm, :],
                            lhsT=w1t_sb[:, kk, mm * P:(mm + 1) * P],
                            rhs=rhs,
                            start=(kk == 0 and mm % 4 == 0),
                            stop=(kk == KO1 - 1),
                            skip_group_check=True,
                        )

            k_group(0, MO, lambda kk: xt[:, kk, :])
            transpose_to(s1, s1T)
            k_group(MO, 2 * MO, lambda kk: s1T[:, kk - MO, :])
            transpose_to(s2, s2T)
            k_group(2 * MO, 3 * MO, lambda kk: s2T[:, kk - 2 * MO, :])

            gT = gpool.tile([P, MO, P], BF16, tag="gT")
            for mm in range(MO):
                nc.scalar.activation(gT[:, mm, :], ps1[:, mm, :], act,
                                     bias=b1s_sb[:, mm:mm + 1])

            # ---- GEMM2 (g^T-stationary) + b2 rank-1 + ACT drain/accum ----
            fg = fgpool.tile([P, H], BF16, tag="fg")
            for nn in range(NCHUNK):
                ps2 = ps2pool.tile([P, CH], F32, tag="ps2")
                for kk in range(KO2):
                    nc.tensor.matmul(
                        ps2,
                        lhsT=gT[:, kk, :],
                        rhs=w2t_sb[:, kk, nn * CH:(nn + 1) * CH],
                        start=(kk == 0),
                        stop=False,
                    )
                nc.tensor.matmul(
                    ps2,
                    lhsT=ones_sb,
                    rhs=b2s_sb[0:1, nn * CH:(nn + 1) * CH],
                    start=False,
                    stop=True,
                )
                nc.scalar.activation(fg[:, nn * CH:(nn + 1) * CH],
                                     ps2, AF.Copy,
                                     accum_out=sacc[:, 4 + nn:5 + nn])
            nc.scalar.activation(dummy, fg, AF.Square,
                                 accum_out=sacc[:, 3:4])

            # ---- LN stats -> mean / rsqrt(var+eps) for x and fg ----
            deriv = spool.tile([P, 8], F32, tag="deriv")
            nc.vector.tensor_add(sacc[:, 1:2], sacc[:, 4:5], sacc[:, 5:6])
            nc.vector.tensor_add(sacc[:, 1:2], sacc[:, 1:2], sacc[:, 6:7])
            # cols 0,1 = mean(x), mean(fg); 2,3 = E[v^2]+eps; 4,5 = mu^2
            nc.vector.tensor_scalar_mul(deriv[:, 0:2], sacc[:, 0:2], 1.0 / H)
            nc.vector.tensor_scalar(deriv[:, 2:4], sacc[:, 2:4],
                                    1.0 / H, EPS, ALU.mult, ALU.add)
            nc.vector.tensor_tensor(deriv[:, 4:6], deriv[:, 0:2],
                                    deriv[:, 0:2], ALU.mult)
            nc.vector.tensor_tensor(deriv[:, 6:8], deriv[:, 2:4],
                                    deriv[:, 4:6], ALU.subtract)
            igt = spool.tile([P, 2], F32, tag="ig")
            nc.vector.reciprocal(igt, deriv[:, 6:8])
            nc.scalar.activation(igt, igt, AF.Sqrt)

            # ---- normalize + sigmoid gate, store ----
            tmp1 = tpool.tile([P, H], BF16, tag="tmp1")
            if uniform_gate:
                # acol = (G1*ig1, G2*ig2); Bc folded via bcs (always 0 here)
                acol = spool.tile([P, 2], F32, tag="acol")
                nc.vector.tensor_tensor(acol, igt, gsc_sb, ALU.mult)
                # tmp1 = (x - mu1) * a1   (4x-mode tensor_scalar)
                nc.vector.tensor_scalar(tmp1, xc, deriv[:, 0:1],
                                        acol[:, 0:1], ALU.subtract, ALU.mult)
                # fg <- (fg - mu2) * a2   (in place)
                nc.vector.tensor_scalar(fg, fg, deriv[:, 1:2],
                                        acol[:, 1:2], ALU.subtract, ALU.mult)
                # tmp1 <- (tmp1 + Bc) + fg
                nc.vector.scalar_tensor_tensor(
                    tmp1, tmp1, bcs_sb[:, 0:1], fg, ALU.add, ALU.add)
            else:
                # tmp1 = (x - mu1) * G1;  G1 = (1-w)*ln1_g  (broadcast)
                nc.vector.scalar_tensor_tensor(
                    tmp1, xc, deriv[:, 0:1], g1b_sb,
                    ALU.subtract, ALU.mult)
                # fg <- (fg - mu2) * G2;  G2 = w*ln2_g   (in place)
                nc.vector.scalar_tensor_tensor(
                    fg, fg, deriv[:, 1:2], g2b_sb,
                    ALU.subtract, ALU.mult)
                # tmp1 = tmp1 * ig1 + Bc;  Bc = w*ln2_b + (1-w)*ln1_b
                nc.vector.scalar_tensor_tensor(
                    tmp1, tmp1, igt[:, 0:1], bcb_sb,
                    ALU.mult, ALU.add)
                # tmp1 <- fg * ig2 + tmp1   (final output)
                nc.vector.scalar_tensor_tensor(
                    tmp1, fg, igt[:, 1:2], tmp1,
                    ALU.mult, ALU.add)
            nc.scalar.dma_start(out_d[t0:t0 + P, :], tmp1)


_NC_CACHE = {}


def _get_nc(uniform_gate=True):
    key = ("nc", uniform_gate)
    if key not in _NC_CACHE:
        _NC_CACHE[key] = _build_program(uniform_gate=uniform_gate)
    return _NC_CACHE[key]


def _host_prep(codes1, scales1, b1, codes2, scales2, b2,
               ln1_g, ln1_b, ln2_g, ln2_b, alpha):
    # W1 with 1/13 folded into the f1/f2 column blocks (mean -> sum)
    w1 = _dequant_nf4(codes1, scales1)
    # match reference rounding: dequant result is cast to bf16 first
    w1 = w1.astype(NP_BF16).astype(np.float32)
    w1[:, H:] *= np.float32(1.0 / 13.0)
    w1t = np.ascontiguousarray(w1.T).astype(NP_BF16)

    w2 = _dequant_nf4(codes2, scales2).astype(NP_BF16)
    w2t = np.ascontiguousarray(w2.astype(np.float32).T).astype(NP_BF16)

    b1s = np.ascontiguousarray(
        b1.astype(np.float32).reshape(MO, P).T)  # [P, MO]
    b2s = np.ascontiguousarray(b2.astype(NP_BF16).reshape(1, H))

    ident = np.eye(P, dtype=NP_BF16)

    a32 = alpha.astype(np.float32)
    w_gate = (1.0 / (1.0 + np.exp(-a32[0]))).astype(NP_BF16)
    one_minus = (NP_BF16(1.0) - w_gate)
    g1 = (one_minus.astype(np.float32) * ln1_g.astype(np.float32))
    g2 = (w_gate.astype(np.float32) * ln2_g.astype(np.float32))
    bc = (w_gate.astype(np.float32) * ln2_b.astype(np.float32)
          + one_minus.astype(np.float32) * ln1_b.astype(np.float32))

    uniform = (np.ptp(g1) == 0.0 and np.ptp(g2) == 0.0 and np.all(bc == 0.0))
    gsc = np.ascontiguousarray(
        np.broadcast_to(np.array([g1[0], g2[0]], np.float32), (P, 2)))
    bcs = np.zeros((P, 2), np.float32)

    g1b = np.ascontiguousarray(np.broadcast_to(g1.astype(NP_BF16), (P, H)))
    g2b = np.ascontiguousarray(np.broadcast_to(g2.astype(NP_BF16), (P, H)))
    bcb = np.ascontiguousarray(np.broadcast_to(bc.astype(NP_BF16), (P, H)))
    return w1t, w2t, b1s, b2s, ident, gsc, bcs, g1b, g2b, bcb, uniform


def _xtc_prep(x):
    """[729, H] token-major -> [NCH, P, MO, P] feature-major token chunks."""
    xT = np.ascontiguousarray(x.T).reshape(MO, P, NT)
    out = np.empty((NCH, P, MO, P), dtype=NP_BF16)
    for c, t0 in enumerate(CHUNK_STARTS):
        out[c] = xT[:, :, t0:t0 + P].transpose(1, 0, 2)
    return out


def make_in_maps(image_features, hidden, codes1, scales1, b1, codes2, scales2,
                 b2, ln1_g, ln1_b, ln2_g, ln2_b, alpha):
    (w1t, w2t, b1s, b2s, ident, gsc, bcs, g1b, g2b, bcb,
     uniform) = _host_prep(codes1, scales1, b1, codes2, scales2, b2,
                           ln1_g, ln1_b, ln2_g, ln2_b, alpha)
    B = image_features.shape[0]
    in_maps = []
    for c in range(B):
        xc = np.ascontiguousarray(image_features[c]).astype(NP_BF16, copy=False)
        in_maps.append({
            "x": xc,
            "xtc": _xtc_prep(xc),
            "hid": np.ascontiguousarray(hidden[:, c]).astype(NP_BF16, copy=False),
            "w1t": w1t, "w2t": w2t, "b1s": b1s, "b2s": b2s, "ident": ident,
            "gsc": gsc, "bcs": bcs,
            "g1b": g1b, "g2b": g2b, "bcb": bcb,
        })
    return in_maps, uniform


def kernel(image_features, hidden, codes1, scales1, b1, codes2, scales2, b2,
           ln1_g, ln1_b, ln2_g, ln2_b, alpha, _trace=False):
    B, N, Hin = image_features.shape
    assert (B, N, Hin) == (8, NT, H), (B, N, Hin)
    in_maps, uniform = make_in_maps(
        image_features, hidden, codes1, scales1, b1, codes2, scales2, b2,
        ln1_g, ln1_b, ln2_g, ln2_b, alpha)
    nc = _get_nc(uniform_gate=uniform)
    res = bass_utils.run_bass_kernel_spmd(
        nc, in_maps, core_ids=list(range(8)), trace=_trace)
    out = np.stack([res.results[c]["out"] for c in range(8)])
    if _trace:
        kernel._last_results = res
    return out.astype(image_features.dtype, copy=False)


# revision 14
# speedup vs baseline: 2.1930x; 1.0066x over previous
"""Trainium2 Bass kernel for nn_Connector_77738908057780 (dense_mlp).

Computation (see reference):
  x   = image_features                      [B, N, H]    bf16
  f1  = mean(hidden[0:13],  axis=0)         [B, N, H]
  f2  = mean(hidden[13:26], axis=0)         [B, N, H]
  cat = concat([x, f1, f2], -1)             [B, N, 3H]
  h   = gelu(cat @ W1.T + b1)               W1 = nf4_dequant(codes1, scales1) [H, 3H]
  fg  = h @ W2.T + b2                       W2 = nf4_dequant(codes2, scales2) [H, H]
  out = w * LN(fg) + (1-w) * LN(x),         w = sigmoid(alpha)

Sharding: data-parallel over batch B=8 -> one batch element per NeuronCore.

Per-core plan (v3 -- chunked pipeline):
  - 6 token chunks of 128 (last chunk overlaps the previous by 39 tokens;
    identical values stored twice -- partial-partition DMA falls off the
    16-engine SDMA path and runs ~15x slower, so all tiles stay full-128).
  - The 26-layer `hidden` stream dominates HBM traffic (46 MB/core); it is
    issued as 12 large 3.8 MB DMAs on the sync HWDGE queue in chunk order so
    DMA stays saturated end-to-end.  Weights stream on the scalar queue
    behind chunk 0's loads.
  - layer sums entirely on DVE (GPSIMD port contention halves DVE throughput
    when both run -- measured), tree-shaped to amortize dispatch and release
    the hid tile early for DMA slot reuse.
  - cat^T is never materialized: GEMM1's k-loop reads x^T (host-transposed
    input), s1^T and s2^T (TensorE identity-transpose -> PSUM -> ACT copy)
    as three separate SBUF tiles.  No SBUF->SBUF xbar DMA at all.
  - GEMM1 weights-stationary -> h^T in PSUM; GELU(+b1 per-partition bias) on
    ACT -> g^T feeds GEMM2 as stationary; b2 is added by a rank-1 matmul
    (ones-row x b2-row) inside the accumulation group; ACT drains PSUM->fg
    while computing sum(fg) via accum_out.
  - LN stats: ACT accum_out gives S(v), S(v^2); DVE combines to mean/var,
    reciprocal+sqrt for rsqrt.  The gate combine uses 4x-mode tensor_scalar
    ops when the folded LN gains are feature-uniform (they are: ln gains are
    ones, biases zeros), falling back to scalar_tensor_tensor otherwise.

NF4 dequant of the (small, replicated) weights is host-side weight prep; the
bf16 weights are less DMA traffic than the int32 codes.
"""

import os
import sys

import numpy as np
import ml_dtypes

for _p in ("/opt/trn_rl_repo", "/root/.axon_site/_ro/trn_rl_repo"):
    if os.path.isdir(_p) and _p not in sys.path:
        sys.path.insert(0, _p)

import concourse.bass as bass
import concourse.mybir as mybir
import concourse.tile as tile
from concourse import bacc
from concourse import bass_utils

BF16 = mybir.dt.bfloat16
F32 = mybir.dt.float32
AF = mybir.ActivationFunctionType
ALU = mybir.AluOpType

NP_BF16 = ml_dtypes.bfloat16

P = 128
H = 1152
H3 = 3456
NT = 729          # tokens per core (N); B=8 cores
L = 26
KO1 = H3 // P     # 27 k-tiles for GEMM1
KO2 = H // P      # 9 k-tiles for GEMM2
MO = H // P       # 9 output-feature tiles
EPS = 1e-5
NCHUNK = 3        # fg free-dim chunks of 384
CH = H // NCHUNK  # 384

# Token chunks; the last starts at 601 so it is a full 128 tokens (tokens
# 601..639 are computed twice with identical values).
CHUNK_STARTS = [0, 128, 256, 384, 512, 601]
NCH = len(CHUNK_STARTS)

NF4_CODEBOOK = np.array([
    -1.0, -0.6961928009986877, -0.5250730514526367, -0.39491748809814453,
    -0.28444138169288635, -0.18477343022823334, -0.09105003625154495, 0.0,
    0.07958029955625534, 0.16093020141124725, 0.24611230194568634,
    0.33791524171829224, 0.4407098591327667, 0.5626170039176941,
    0.7229568362236023, 1.0], dtype=np.float32)

BLOCK = 64


def _dequant_nf4(codes, scales):
    """Match reference: codebook lookup * per-64-block absmax, cast bf16."""
    out_f, in_f = codes.shape
    w = NF4_CODEBOOK[codes].reshape(out_f, in_f // BLOCK, BLOCK)
    w = w * scales[:, :, None].astype(np.float32)
    return w.reshape(out_f, in_f)  # float32 (caller casts)


def _build_program(act=AF.Gelu, uniform_gate=True):
    nc = bacc.Bacc(
        "TRN2",
        target_bir_lowering=False,
        debug=False,
        num_devices=1,
    )
    x_d = nc.dram_tensor("x", (NT, H), BF16, kind="ExternalInput").ap()
    xtc_d = nc.dram_tensor("xtc", (NCH, P, MO, P), BF16, kind="ExternalInput").ap()
    hid_d = nc.dram_tensor("hid", (L, NT, H), BF16, kind="ExternalInput").ap()
    w1t_d = nc.dram_tensor("w1t", (H3, H), BF16, kind="ExternalInput").ap()
    w2t_d = nc.dram_tensor("w2t", (H, H), BF16, kind="ExternalInput").ap()
    b1s_d = nc.dram_tensor("b1s", (P, MO), F32, kind="ExternalInput").ap()
    b2s_d = nc.dram_tensor("b2s", (1, H), BF16, kind="ExternalInput").ap()
    ident_d = nc.dram_tensor("ident", (P, P), BF16, kind="ExternalInput").ap()
    # uniform path: gsc = per-partition (G1, G2) scalars; bcs = (Bc, 0)
    gsc_d = nc.dram_tensor("gsc", (P, 2), F32, kind="ExternalInput").ap()
    bcs_d = nc.dram_tensor("bcs", (P, 2), F32, kind="ExternalInput").ap()
    # general path: per-feature broadcasts
    g1b_d = nc.dram_tensor("g1b", (P, H), BF16, kind="ExternalInput").ap()
    g2b_d = nc.dram_tensor("g2b", (P, H), BF16, kind="ExternalInput").ap()
    bcb_d = nc.dram_tensor("bcb", (P, H), BF16, kind="ExternalInput").ap()
    out_d = nc.dram_tensor("out", (NT, H), BF16, kind="ExternalOutput").ap()

    with tile.TileContext(nc) as tc:
        _program(nc, tc, x_d, xtc_d, hid_d, w1t_d, w2t_d, b1s_d, b2s_d,
                 ident_d, gsc_d, bcs_d, g1b_d, g2b_d, bcb_d, out_d, act,
                 uniform_gate)

    nc.compile()
    return nc


def _program(nc, tc, x_d, xtc_d, hid_d, w1t_d, w2t_d, b1s_d, b2s_d, ident_d,
             gsc_d, bcs_d, g1b_d, g2b_d, bcb_d, out_d, act, uniform_gate):
    with (
        tc.tile_pool(name="consts", bufs=1) as cpool,
        tc.tile_pool(name="hid", bufs=4) as hpool,
        tc.tile_pool(name="xt", bufs=2) as xtpool,
        tc.tile_pool(name="x", bufs=3) as xpool,
        tc.tile_pool(name="scr", bufs=1) as scrpool,
        tc.tile_pool(name="acc", bufs=2) as apool,
        tc.tile_pool(name="st", bufs=2) as stpool,
        tc.tile_pool(name="g", bufs=2) as gpool,
        tc.tile_pool(name="fg", bufs=2) as fgpool,
        tc.tile_pool(name="tmp", bufs=2) as tpool,
        tc.tile_pool(name="dum", bufs=1) as dpool,
        tc.tile_pool(name="stats", bufs=2) as spool,
        tc.tile_pool(name="ps1", bufs=1, space="PSUM") as ps1pool,
        tc.tile_pool(name="ps2", bufs=3, space="PSUM") as ps2pool,
        tc.tile_pool(name="pt", bufs=2, space="PSUM") as ptpool,
    ):
        # ---- small constants first (sync queue; ~50 KB total) ----
        b1s_sb = cpool.tile([P, MO], F32)
        nc.sync.dma_start(b1s_sb, b1s_d)
        b2s_sb = cpool.tile([1, H], BF16)
        nc.sync.dma_start(b2s_sb, b2s_d)
        ident_sb = cpool.tile([P, P], BF16)
        nc.sync.dma_start(ident_sb, ident_d)
        if uniform_gate:
            gsc_sb = cpool.tile([P, 2], F32)
            nc.sync.dma_start(gsc_sb, gsc_d)
            bcs_sb = cpool.tile([P, 2], F32)
            nc.sync.dma_start(bcs_sb, bcs_d)
        else:
            g1b_sb = cpool.tile([P, H], BF16)
            nc.sync.dma_start(g1b_sb, g1b_d)
            g2b_sb = cpool.tile([P, H], BF16)
            nc.sync.dma_start(g2b_sb, g2b_d)
            bcb_sb = cpool.tile([P, H], BF16)
            nc.sync.dma_start(bcb_sb, bcb_d)
        ones_sb = cpool.tile([1, P], BF16)
        nc.vector.memset(ones_sb, 1.0)

        w1t_sb = cpool.tile([P, KO1, H], BF16)
        w2t_sb = cpool.tile([P, KO2, H], BF16)
        w1t_r = w1t_d.rearrange("(ko p) n -> p ko n", p=P)

        dummy = dpool.tile([P, H], BF16, tag="dummy")
        # DVE-serial scratch for the layer-sum trees (reused across chunks)
        scr = [scrpool.tile([P, 3, H], BF16, name=f"scr{i}", tag=f"scr{i}")
               for i in range(2)]

        def half_sum(h7, h6, dst, scr):
            """dst[t, f] = sum over the 7-layer and 6-layer pieces, on DVE.

            Tree-shaped to amortize DVE dispatch; each hid piece is fully
            consumed after two ops so its DMA slot recycles early."""
            t7 = scr[0]
            nc.vector.tensor_add(t7, h7[:, 0:3, :], h7[:, 3:6, :])
            nc.vector.tensor_add(t7[:, 2, :], t7[:, 2, :], h7[:, 6, :])
            t6 = scr[1]
            nc.vector.tensor_add(t6, h6[:, 0:3, :], h6[:, 3:6, :])
            nc.vector.tensor_add(t7, t7, t6)
            nc.vector.tensor_add(dst, t7[:, 0, :], t7[:, 1, :])
            nc.vector.tensor_add(dst, dst, t7[:, 2, :])

        def transpose_to(src, dst):
            """src [P, H] token-major -> dst [P, MO, P] feature-major."""
            for g0 in (0, 4, 8):
                g = min(4, MO - g0)
                pt = ptpool.tile([P, 4, P], BF16, tag="pt")
                for j in range(g):
                    nc.tensor.transpose(
                        pt[:, j, :],
                        src[:, (g0 + j) * P:(g0 + j + 1) * P],
                        ident_sb)
                nc.scalar.activation(dst[:, g0:g0 + g, :],
                                     pt[:, 0:g, :], AF.Copy)

        for c, t0 in enumerate(CHUNK_STARTS):
            # ---- DMA issues (loads only; stores go at the chunk end) ----
            hps = []
            for l0, nl in ((0, 7), (7, 6), (13, 7), (20, 6)):
                hp = hpool.tile([P, 7, H], BF16, tag="hid")
                nc.sync.dma_start(
                    hp[:, 0:nl, :],
                    hid_d[l0:l0 + nl, t0:t0 + P, :].rearrange(
                        "l p f -> p l f"))
                hps.append(hp)
            xt = xtpool.tile([P, MO, P], BF16, tag="xtc")
            nc.scalar.dma_start(xt, xtc_d[c])
            if c == 0:
                # weights stream behind chunk 0's x^T on the scalar queue,
                # ordered so GEMM1's k-outer loop can start early
                nc.scalar.dma_start(w1t_sb[:, 0:9, :], w1t_r[:, 0:9, :])
            xc = xpool.tile([P, H], BF16, tag="x")
            nc.scalar.dma_start(xc, x_d[t0:t0 + P, :])
            if c == 0:
                nc.scalar.dma_start(w1t_sb[:, 9:18, :], w1t_r[:, 9:18, :])
                nc.scalar.dma_start(w1t_sb[:, 18:27, :], w1t_r[:, 18:27, :])
                nc.scalar.dma_start(
                    w2t_sb, w2t_d.rearrange("(ko p) n -> p ko n", p=P))

            # ---- 13-layer sums on DVE ----
            s1 = apool.tile([P, H], BF16, tag="s1")
            half_sum(hps[0], hps[1], s1, scr)
            s2 = apool.tile([P, H], BF16, tag="s2")
            half_sum(hps[2], hps[3], s2, scr)

            sacc = spool.tile([P, 8], F32, tag="sacc")

            # ---- GEMM1 (weights-stationary, k-outer) with s1/s2 transposes
            # (TensorE identity -> PSUM -> ACT copy) issued ahead of the
            # k-groups that consume them, so the ACT copies hide under the
            # preceding matmuls.  The final k-group is m-outer so each GELU
            # fires as soon as its m-tile finishes.
            ps1 = ps1pool.tile([P, MO, P], F32, tag="ps1")
            s1T = stpool.tile([P, MO, P], BF16, tag="s1T")
            s2T = stpool.tile([P, MO, P], BF16, tag="s2T")
            gT = gpool.tile([P, MO, P], BF16, tag="gT")

            def mm1(kk, mm, rhs):
                # start=True marks the whole 2KB PSUM bank pending-zero, so
                # only the first matmul touching each bank sets it; the
                # other m-slices' first writes land on still-pending bytes
                # and overwrite (HW has_written semantics; sim mirrors it).
                nc.tensor.matmul(
                    ps1[:, mm, :],
                    lhsT=w1t_sb[:, kk, mm * P:(mm + 1) * P],
                    rhs=rhs,
                    start=(kk == 0 and mm % 4 == 0),
                    stop=(kk == KO1 - 1),
                    skip_group_check=True,
                )

            transpose_to(s1, s1T)
            for kk in range(0, MO):
                for mm in range(MO):
                    mm1(kk, mm, xt[:, kk, :])
            transpose_to(s2, s2T)
            for kk in range(MO, 2 * MO):
                for mm in range(MO):
                    mm1(kk, mm, s1T[:, kk - MO, :])
            for mm in range(MO):
                for kk in range(2 * MO, 3 * MO):
                    mm1(kk, mm, s2T[:, kk - 2 * MO, :])
                nc.scalar.activation(gT[:, mm, :], ps1[:, mm, :], act,
                                     bias=b1s_sb[:, mm:mm + 1])

            # ---- LN1(x) raw sums on ACT (fill the GELU->drain gap) ----
            nc.scalar.activation(dummy, xc, AF.Copy,
                                 accum_out=sacc[:, 0:1])
            nc.scalar.activation(dummy, xc, AF.Square,
                                 accum_out=sacc[:, 2:3])

            # ---- GEMM2 (g^T-stationary, k-outer) + b2 rank-1 + ACT drain --
            fg = fgpool.tile([P, H], BF16, tag="fg")
            ps2s = [ps2pool.tile([P, CH], F32, name=f"ps2_{nn}", tag="ps2")
                    for nn in range(NCHUNK)]
            for kk in range(KO2):
                for nn in range(NCHUNK):
                    nc.tensor.matmul(
                        ps2s[nn],
                        lhsT=gT[:, kk, :],
                        rhs=w2t_sb[:, kk, nn * CH:(nn + 1) * CH],
                        start=(kk == 0),
                        stop=False,
                    )
            for nn in range(NCHUNK):
                nc.tensor.matmul(
                    ps2s[nn],
                    lhsT=ones_sb,
                    rhs=b2s_sb[0:1, nn * CH:(nn + 1) * CH],
                    start=False,
                    stop=True,
                )
                nc.scalar.activation(fg[:, nn * CH:(nn + 1) * CH],
                                     ps2s[nn], AF.Copy,
                                     accum_out=sacc[:, 4 + nn:5 + nn])
            nc.scalar.activation(dummy, fg, AF.Square,
                                 accum_out=sacc[:, 3:4])

            # ---- LN stats -> mean / rsqrt(var+eps) for x and fg ----
            deriv = spool.tile([P, 8], F32, tag="deriv")
            nc.vector.tensor_add(sacc[:, 1:2], sacc[:, 4:5], sacc[:, 5:6])
            nc.vector.tensor_add(sacc[:, 1:2], sacc[:, 1:2], sacc[:, 6:7])
            # cols 0,1 = mean(x), mean(fg); 2,3 = E[v^2]+eps; 4,5 = mu^2
            nc.vector.tensor_scalar_mul(deriv[:, 0:2], sacc[:, 0:2], 1.0 / H)
            nc.vector.tensor_scalar(deriv[:, 2:4], sacc[:, 2:4],
                                    1.0 / H, EPS, ALU.mult, ALU.add)
            nc.vector.tensor_tensor(deriv[:, 4:6], deriv[:, 0:2],
                                    deriv[:, 0:2], ALU.mult)
            nc.vector.tensor_tensor(deriv[:, 6:8], deriv[:, 2:4],
                                    deriv[:, 4:6], ALU.subtract)
            igt = spool.tile([P, 2], F32, tag="ig")
            nc.vector.reciprocal(igt, deriv[:, 6:8])
            nc.scalar.activation(igt, igt, AF.Sqrt)

            # ---- normalize + sigmoid gate, store ----
            tmp1 = tpool.tile([P, H], BF16, tag="tmp1")
            if uniform_gate:
                # acol = (G1*ig1, G2*ig2); Bc folded via bcs (always 0 here)
                acol = spool.tile([P, 2], F32, tag="acol")
                nc.vector.tensor_tensor(acol, igt, gsc_sb, ALU.mult)
                # tmp1 = (x - mu1) * a1   (4x-mode tensor_scalar)
                nc.vector.tensor_scalar(tmp1, xc, deriv[:, 0:1],
                                        acol[:, 0:1], ALU.subtract, ALU.mult)
                # fg <- (fg - mu2) * a2   (in place)
                nc.vector.tensor_scalar(fg, fg, deriv[:, 1:2],
                                        acol[:, 1:2], ALU.subtract, ALU.mult)
                # tmp1 <- (tmp1 + Bc) + fg
                nc.vector.scalar_tensor_tensor(
                    tmp1, tmp1, bcs_sb[:, 0:1], fg, ALU.add, ALU.add)
            else:
                # tmp1 = (x - mu1) * G1;  G1 = (1-w)*ln1_g  (broadcast)
                nc.vector.scalar_tensor_tensor(
                    tmp1, xc, deriv[:, 0:1], g1b_sb,
                    ALU.subtract, ALU.mult)
                # fg <- (fg - mu2) * G2;  G2 = w*ln2_g   (in place)
                nc.vector.scalar_tensor_tensor(
                    fg, fg, deriv[:, 1:2], g2b_sb,
                    ALU.subtract, ALU.mult)
                # tmp1 = tmp1 * ig1 + Bc;  Bc = w*ln2_b + (1-w)*ln1_b
                nc.vector.scalar_tensor_tensor(
                    tmp1, tmp1, igt[:, 0:1], bcb_sb,
                    ALU.mult, ALU.add)
                # tmp1 <- fg * ig2 + tmp1   (final output)
                nc.vector.scalar_tensor_tensor(
                    tmp1, fg, igt[:, 1:2], tmp1,
                    ALU.mult, ALU.add)
            nc.scalar.dma_start(out_d[t0:t0 + P, :], tmp1)


_NC_CACHE = {}


def _get_nc(uniform_gate=True):
    key = ("nc", uniform_gate)
    if key not in _NC_CACHE:
        _NC_CACHE[key] = _build_program(uniform_gate=uniform_gate)
    return _NC_CACHE[key]


def _host_prep(codes1, scales1, b1, codes2, scales2, b2,
               ln1_g, ln1_b, ln2_g, ln2_b, alpha):
    # W1 with 1/13 folded into the f1/f2 column blocks (mean -> sum)
    w1 = _dequant_nf4(codes1, scales1)
    # match reference rounding: dequant result is cast to bf16 first
    w1 = w1.astype(NP_BF16).astype(np.float32)
    w1[:, H:] *= np.float32(1.0 / 13.0)
    w1t = np.ascontiguousarray(w1.T).astype(NP_BF16)

    w2 = _dequant_nf4(codes2, scales2).astype(NP_BF16)
    w2t = np.ascontiguousarray(w2.astype(np.float32).T).astype(NP_BF16)

    b1s = np.ascontiguousarray(
        b1.astype(np.float32).reshape(MO, P).T)  # [P, MO]
    b2s = np.ascontiguousarray(b2.astype(NP_BF16).reshape(1, H))

    ident = np.eye(P, dtype=NP_BF16)

    a32 = alpha.astype(np.float32)
    w_gate = (1.0 / (1.0 + np.exp(-a32[0]))).astype(NP_BF16)
    one_minus = (NP_BF16(1.0) - w_gate)
    g1 = (one_minus.astype(np.float32) * ln1_g.astype(np.float32))
    g2 = (w_gate.astype(np.float32) * ln2_g.astype(np.float32))
    bc = (w_gate.astype(np.float32) * ln2_b.astype(np.float32)
          + one_minus.astype(np.float32) * ln1_b.astype(np.float32))

    uniform = (np.ptp(g1) == 0.0 and np.ptp(g2) == 0.0 and np.all(bc == 0.0))
    gsc = np.ascontiguousarray(
        np.broadcast_to(np.array([g1[0], g2[0]], np.float32), (P, 2)))
    bcs = np.zeros((P, 2), np.float32)

    g1b = np.ascontiguousarray(np.broadcast_to(g1.astype(NP_BF16), (P, H)))
    g2b = np.ascontiguousarray(np.broadcast_to(g2.astype(NP_BF16), (P, H)))
    bcb = np.ascontiguousarray(np.broadcast_to(bc.astype(NP_BF16), (P, H)))
    return w1t, w2t, b1s, b2s, ident, gsc, bcs, g1b, g2b, bcb, uniform


def _xtc_prep(x):
    """[729, H] token-major -> [NCH, P, MO, P] feature-major token chunks."""
    xT = np.ascontiguousarray(x.T).reshape(MO, P, NT)
    out = np.empty((NCH, P, MO, P), dtype=NP_BF16)
    for c, t0 in enumerate(CHUNK_STARTS):
        out[c] = xT[:, :, t0:t0 + P].transpose(1, 0, 2)
    return out


def make_in_maps(image_features, hidden, codes1, scales1, b1, codes2, scales2,
                 b2, ln1_g, ln1_b, ln2_g, ln2_b, alpha):
    (w1t, w2t, b1s, b2s, ident, gsc, bcs, g1b, g2b, bcb,
     uniform) = _host_prep(codes1, scales1, b1, codes2, scales2, b2,
                           ln1_g, ln1_b, ln2_g, ln2_b, alpha)
    B = image_features.shape[0]
    in_maps = []
    for c in range(B):
        xc = np.ascontiguousarray(image_features[c]).astype(NP_BF16, copy=False)
        in_maps.append({
            "x": xc,
            "xtc": _xtc_prep(xc),
            "hid": np.ascontiguousarray(hidden[:, c]).astype(NP_BF16, copy=False),
            "w1t": w1t, "w2t": w2t, "b1s": b1s, "b2s": b2s, "ident": ident,
            "gsc": gsc, "bcs": bcs,
            "g1b": g1b, "g2b": g2b, "bcb": bcb,
        })
    return in_maps, uniform


def kernel(image_features, hidden, codes1, scales1, b1, codes2, scales2, b2,
           ln1_g, ln1_b, ln2_g, ln2_b, alpha, _trace=False):
    B, N, Hin = image_features.shape
    assert (B, N, Hin) == (8, NT, H), (B, N, Hin)
    in_maps, uniform = make_in_maps(
        image_features, hidden, codes1, scales1, b1, codes2, scales2, b2,
        ln1_g, ln1_b, ln2_g, ln2_b, alpha)
    nc = _get_nc(uniform_gate=uniform)
    res = bass_utils.run_bass_kernel_spmd(
        nc, in_maps, core_ids=list(range(8)), trace=_trace)
    out = np.stack([res.results[c]["out"] for c in range(8)])
    if _trace:
        kernel._last_results = res
    return out.astype(image_features.dtype, copy=False)
